# revision 44
# baseline (speedup 1.0000x reference)
"""Trainium2 Bass kernel for nn_DecoderModel_54795192762653.

4-layer decoder, B=4, T=1024, D=1024, H=16, K=4 kv-heads, HD=64, F=4096,
V=32000. 8 NeuronCores: pair (2b, 2b+1) handles batch b; within a pair,
core A owns tokens 0..511 and core B owns 512..1023.

v3 changes vs v2:
- single merged pair-AllGather per layer (k + v + [layer-3: q-rows + h_1023])
- ReduceScatter split in two (rout 0-3 / 4-7) to overlap wire with matmuls
- attention o-accumulators evacuated raw to SBUF (fast PSUM free), softmax
  normalization deferred off the critical path
- full-shape contiguous causal masks (8 variants), DVE multiply
- last layer specialized: only token 1023 survives the block, so layer 3
  computes k/v for all tokens plus a 16-row q slice, a 1-token attention,
  a 1-token out-projection, and a pair-F-sharded 1-token FFN; partial sums
  meet in the final 8-core AllGather
- LM head weights half-preloaded into SBUF at kernel start

Attention uses the reference's "scrambled" reshape semantics: unit m
(m = g*4 + kv) reads q rows m*64..(m+1)*64 (all channels) viewed as
(1024 l x 64 d); k/v block c = m % 4 rows c*256..(c+1)*256 viewed as
(1024 j x 64 d). Scores are computed transposed (j on partitions,
j = 4*(token offset in c-block) + h4), l = 16*tau + 2*hidx + par.
Softmax denominator comes from a ones-column appended to V (M=65 matmul).
"""
import sys

sys.path.insert(0, "/opt/trn_rl_repo")

import numpy as np
import ml_dtypes
from contextlib import ExitStack

import concourse.bass as bass
import concourse.tile as tile
from concourse import bacc, mybir
from concourse.bass_utils import run_bass_kernel_spmd

P = 128
F32 = mybir.dt.float32
F32R = mybir.dt.float32r
BF16 = mybir.dt.bfloat16
U32 = mybir.dt.uint32
AF = mybir.ActivationFunctionType
OP = mybir.AluOpType
NPBF16 = ml_dtypes.bfloat16

D, H, KV, F, L, V, T, B = 1024, 16, 4, 4096, 4, 32000, 1024, 4
HD = D // H
TL = T // 2          # 512 tokens per core
VC = V // 8          # 4000 vocab per core
EPS = 1e-5
PAIRS = [[0, 1], [2, 3], [4, 5], [6, 7]]
ALL8 = [list(range(8))]

# merged AG buffer regions (bf16 elements)
KOFF, VOFF, QOFF, HOFF, AGN = 0, 131072, 262144, 262656, 263680

_CACHE = {}


def _layer_norm(nc, pools, h_tiles, out_tiles, g_ap, b_ap):
    """Feature-major layernorm over D=1024 (8 partition tiles x 512 tokens)."""
    wk, ps_mm, ones_col = pools["wk"], pools["ps_mm"], pools["ones_col"]
    s1 = ps_mm.tile([P, 512], F32, name="mm")
    s2 = ps_mm.tile([P, 512], F32, name="mm")
    for r in range(8):
        nc.tensor.matmul(s1[0:1, :], ones_col[:, 0:1], h_tiles[r],
                         start=(r == 0), stop=(r == 7))
    for r in range(8):
        sq = wk.tile([P, 512], F32R, name="ln_sq")
        nc.scalar.activation(sq[:], h_tiles[r], AF.Square)
        nc.tensor.matmul(s2[0:1, :], ones_col[:, 0:1], sq[:],
                         start=(r == 0), stop=(r == 7))
    mu = wk.tile([1, 512], F32, name="ln_mu")
    nc.scalar.mul(mu[:], s1[0:1, :], 1.0 / D)
    e2 = wk.tile([1, 512], F32, name="ln_e2")
    nc.scalar.mul(e2[:], s2[0:1, :], 1.0 / D)
    musq = wk.tile([1, 512], F32, name="ln_musq")
    nc.scalar.activation(musq[:], mu[:], AF.Square)
    var = wk.tile([1, 512], F32, name="ln_var")
    nc.vector.tensor_sub(var[:], e2[:], musq[:])
    sd = wk.tile([1, 512], F32, name="ln_sd")
    nc.scalar.activation(sd[:], var[:], AF.Sqrt, bias=pools["eps"][0:1, :])
    rv = wk.tile([1, 512], F32, name="ln_rv")
    nc.vector.reciprocal(rv[:], sd[:])
    cv = wk.tile([1, 512], F32, name="ln_cv")
    nc.vector.scalar_tensor_tensor(cv[:], mu[:], -1.0, rv[:],
                                   op0=OP.mult, op1=OP.mult)
    rb = wk.tile([P, 512], F32, name="ln_rb")
    nc.gpsimd.partition_broadcast(rb[:], rv[:])
    cb = wk.tile([P, 512], F32, name="ln_cb")
    nc.gpsimd.partition_broadcast(cb[:], cv[:])
    for r in range(8):
        t1 = wk.tile([P, 512], F32, name="ln_t1")
        nc.vector.tensor_mul(t1[:], h_tiles[r], rb[:])
        nc.vector.tensor_add(t1[:], t1[:], cb[:])
        nc.scalar.activation(out_tiles[r], t1[:], AF.Identity,
                             bias=b_ap(r), scale=g_ap(r))


def _vec_ln(nc, pools, pool, src2, out_fn, g_ap, b_ap, nb):
    """Feature-major layernorm of nb token columns. src2: [P, 8, nb] F32R
    (nb even); writes out via out_fn(r) -> [P, nb] APs (may be bf16)."""
    ps_mm, ones_col, eps_t = pools["ps_mm"], pools["ones_col"], pools["eps"]
    s1 = ps_mm.tile([P, 512], F32, name="mm")
    s2 = ps_mm.tile([P, 512], F32, name="mm")
    for r in range(8):
        nc.tensor.matmul(s1[0:1, 0:nb], ones_col[:, 0:1], src2[:, r, :],
                         start=(r == 0), stop=(r == 7))
    for r in range(8):
        sqf = pool.tile([P, 8], F32R, name="vln_sq")
        nc.scalar.activation(sqf[:, 0:nb], src2[:, r, :], AF.Square)
        nc.tensor.matmul(s2[0:1, 0:nb], ones_col[:, 0:1], sqf[:, 0:nb],
                         start=(r == 0), stop=(r == 7))
    mu = pool.tile([1, 8], F32, name="vln_mu")
    nc.scalar.mul(mu[0:1, 0:nb], s1[0:1, 0:nb], 1.0 / D)
    e2 = pool.tile([1, 8], F32, name="vln_e2")
    nc.scalar.mul(e2[0:1, 0:nb], s2[0:1, 0:nb], 1.0 / D)
    musq = pool.tile([1, 8], F32, name="vln_musq")
    nc.scalar.activation(musq[0:1, 0:nb], mu[0:1, 0:nb], AF.Square)
    var = pool.tile([1, 8], F32, name="vln_var")
    nc.vector.tensor_sub(var[0:1, 0:nb], e2[0:1, 0:nb], musq[0:1, 0:nb])
    sd = pool.tile([1, 8], F32, name="vln_sd")
    nc.scalar.activation(sd[0:1, 0:nb], var[0:1, 0:nb], AF.Sqrt,
                         bias=eps_t[0:1, :])
    rv = pool.tile([1, 8], F32, name="vln_rv")
    nc.vector.reciprocal(rv[0:1, 0:nb], sd[0:1, 0:nb])
    cv = pool.tile([1, 8], F32, name="vln_cv")
    nc.vector.scalar_tensor_tensor(cv[0:1, 0:nb], mu[0:1, 0:nb], -1.0,
                                   rv[0:1, 0:nb], op0=OP.mult, op1=OP.mult)
    rb = pool.tile([P, 8], F32, name="vln_rb")
    nc.gpsimd.partition_broadcast(rb[:, 0:nb], rv[0:1, 0:nb])
    cb = pool.tile([P, 8], F32, name="vln_cb")
    nc.gpsimd.partition_broadcast(cb[:, 0:nb], cv[0:1, 0:nb])
    for r in range(8):
        t1 = pool.tile([P, 8], F32, name="vln_t1")
        nc.vector.tensor_mul(t1[:, 0:nb], src2[:, r, :], rb[:, 0:nb])
        nc.vector.tensor_add(t1[:, 0:nb], t1[:, 0:nb], cb[:, 0:nb])
        nc.scalar.activation(out_fn(r), t1[:, 0:nb], AF.Identity,
                             bias=b_ap(r), scale=g_ap(r))


def build_kernel(n_layers=L):
    nc = bacc.Bacc("TRN2", target_bir_lowering=False, debug=False, num_devices=8)

    # ---------------- I/O ----------------
    h0t_d = nc.dram_tensor("h0t", [D, TL], F32R, kind="ExternalInput")
    wqkv_d = nc.dram_tensor("wqkv", [n_layers, 10, P, 8, P], BF16,
                            kind="ExternalInput")
    wv_d = nc.dram_tensor("wv", [n_layers, P, 8, 256], BF16,
                          kind="ExternalInput")
    wout_d = nc.dram_tensor("wout", [n_layers, 8, P, 4, P], BF16,
                            kind="ExternalInput")
    wout3_d = nc.dram_tensor("wout3", [8, P, 8, P], BF16, kind="ExternalInput")
    wup_d = nc.dram_tensor("wup", [n_layers, 16, P, 8, 256], BF16,
                           kind="ExternalInput")
    wgate_d = nc.dram_tensor("wgate", [n_layers, 16, P, 8, 256], BF16,
                             kind="ExternalInput")
    wdown_d = nc.dram_tensor("wdown", [n_layers, F, D], BF16,
                             kind="ExternalInput")
    wup3_d = nc.dram_tensor("wup3", [16, P, 8, P], BF16, kind="ExternalInput")
    wgate3_d = nc.dram_tensor("wgate3", [16, P, 8, P], BF16,
                              kind="ExternalInput")
    wdown3_d = nc.dram_tensor("wdown3", [8, P, 16, P], BF16,
                              kind="ExternalInput")
    bup3_d = nc.dram_tensor("bup3", [P, 16], F32, kind="ExternalInput")
    bgate3_d = nc.dram_tensor("bgate3", [P, 16], F32, kind="ExternalInput")
    ln1g_d = nc.dram_tensor("ln1g", [n_layers, D], F32, kind="ExternalInput")
    ln1b_d = nc.dram_tensor("ln1b", [n_layers, D], F32, kind="ExternalInput")
    ln2g_d = nc.dram_tensor("ln2g", [n_layers, D], F32, kind="ExternalInput")
    ln2b_d = nc.dram_tensor("ln2b", [n_layers, D], F32, kind="ExternalInput")
    bup_d = nc.dram_tensor("bup", [n_layers, F], F32, kind="ExternalInput")
    bgate_d = nc.dram_tensor("bgate", [n_layers, F], F32, kind="ExternalInput")
    bdown_d = nc.dram_tensor("bdown", [n_layers, D], F32, kind="ExternalInput")
    flng_d = nc.dram_tensor("flng", [D], F32, kind="ExternalInput")
    flnb_d = nc.dram_tensor("flnb", [D], F32, kind="ExternalInput")
    wlm_d = nc.dram_tensor("wlm", [P, 8, VC], BF16, kind="ExternalInput")
    blm_d = nc.dram_tensor("blm", [VC], F32, kind="ExternalInput")
    logits_d = nc.dram_tensor("logits", [B, VC], F32, kind="ExternalOutput")

    # collective bounce buffers (internal DRAM)
    kv_ag_in = nc.dram_tensor("kv_ag_in", [AGN], BF16)
    kv_ag_out = nc.dram_tensor("kv_ag_out", [2, AGN], BF16)
    rs_in = nc.dram_tensor("rs_in", [2, 8, P, TL], BF16)      # [half, rout]
    rs_out = nc.dram_tensor("rs_out", [8, P, TL], BF16)
    # tiny dummy collectives to warm up ncfw for both replica-group shapes
    wu_ag_in = nc.dram_tensor("wu_ag_in", [64], BF16)
    wu_ag_out = nc.dram_tensor("wu_ag_out", [2, 64], BF16)
    wu8_ag_in = nc.dram_tensor("wu8_ag_in", [64], BF16)
    wu8_ag_out = nc.dram_tensor("wu8_ag_out", [8, 64], BF16)
    fin_ag_in = nc.dram_tensor("fin_ag_in", [D], F32)
    fin_ag_out = nc.dram_tensor("fin_ag_out", [8, D], F32, addr_space="Shared")

    with tile.TileContext(nc) as tc, ExitStack() as ctx:
        pers = ctx.enter_context(tc.tile_pool(name="pers", bufs=1))
        wk = ctx.enter_context(tc.tile_pool(name="wk", bufs=2))
        ps_mm = ctx.enter_context(tc.tile_pool(name="ps_mm", bufs=3, space="PSUM"))
        ps_acc = ctx.enter_context(tc.tile_pool(name="ps_acc", bufs=1, space="PSUM"))
        ps_dum = ctx.enter_context(tc.tile_pool(name="ps_dum", bufs=1,
                                                space="PSUM"))
        pools = {"wk": wk, "ps_mm": ps_mm}

        # initial residual first in the DMA queue: LN1 of layer 0 needs it
        h = pers.tile([P, 8, 512], F32R, tag="h")      # residual stream h^T
        nc.sync.dma_start(h[:], h0t_d.ap().rearrange("(kt p) t -> p kt t", p=P))

        # warm up ncfw for both replica-group shapes while startup DMAs fly
        with tc.high_priority():
            nc.gpsimd.collective_compute(
                "AllGather", OP.bypass, replica_groups=PAIRS,
                ins=[wu_ag_in[:]], outs=[wu_ag_out[:, :]])
            nc.gpsimd.collective_compute(
                "AllGather", OP.bypass, replica_groups=ALL8,
                ins=[wu8_ag_in[:]], outs=[wu8_ag_out[:, :]])

        # ---------------- constants ----------------
        ones_col = pers.tile([P, 1], F32R, tag="ones_col")
        nc.gpsimd.memset(ones_col[:].bitcast(F32), 1.0)
        pools["ones_col"] = ones_col
        eps_t = pers.tile([P, 1], F32, tag="eps")
        nc.gpsimd.memset(eps_t[:], EPS)
        pools["eps"] = eps_t

        # keep-warm dummies: hold the PE HAM clock-gate open across the
        # short exp->mask waits inside attention
        dum_ps = ps_dum.tile([P, 512], F32, tag="dum")

        def warm(n):
            for _ in range(n):
                nc.tensor.matmul(dum_ps[0:1, :], ones_col[:, 0:1], h[:, 0, :],
                                 start=True, stop=True)

        # causal masks: keep iff l - j >= 0 with
        # l = 16*tau + 2*hidx + par, j = 512*tlt + 4*p + h4
        masks = []
        with ExitStack() as mctx:
            mpool = mctx.enter_context(tc.tile_pool(name="maskinit", bufs=2))
            for h4 in range(4):
                mf = mpool.tile([P, 2, 8, 2, 32], F32, name="maskf")
                nc.gpsimd.memset(mf[:], 1.0)
                nc.gpsimd.affine_select(
                    out=mf[:], in_=mf[:],
                    pattern=[[1, 2], [2, 8], [0, 2], [16, 32]],
                    channel_multiplier=-4, base=-h4,
                    compare_op=OP.is_ge, fill=0.0)
                mb = pers.tile([P, 2, 8, 2, 32], BF16, tag=f"maskb{h4}")
                nc.vector.tensor_copy(mb[:], mf[:])
                masks.append(mb)

        # LM head: preload first quarter (nt 0-1) into SBUF
        wlmA = pers.tile([P, 8, 2000], BF16, tag="wlmA")
        for kt in range(8):
            nc.sync.dma_start(wlmA[:, kt, :], wlm_d[:, kt, 0:2000])

        # ---------------- per-layer params (small, load all) ----------------
        lnp = {}
        for name, dram, nt in [("ln1g", ln1g_d, 8), ("ln1b", ln1b_d, 8),
                               ("ln2g", ln2g_d, 8), ("ln2b", ln2b_d, 8),
                               ("bup", bup_d, 32), ("bgate", bgate_d, 32),
                               ("bdown", bdown_d, 8)]:
            t = pers.tile([P, n_layers, nt], F32, tag=f"p_{name}")
            nc.sync.dma_start(t[:], dram.ap().rearrange("l (t p) -> p l t", p=P))
            lnp[name] = t
        fln = pers.tile([P, 2, 8], F32, tag="p_fln")
        nc.sync.dma_start(fln[:, 0], flng_d.ap().rearrange("(t p) -> p t", p=P))
        nc.sync.dma_start(fln[:, 1], flnb_d.ap().rearrange("(t p) -> p t", p=P))
        b3 = pers.tile([P, 2, 16], F32, tag="p_b3")
        nc.sync.dma_start(b3[:, 0], bup3_d[:, :])
        nc.sync.dma_start(b3[:, 1], bgate3_d[:, :])

        # ---------------- layers 0..n-2 (full) ----------------
        for ly in range(n_layers):
            last = (ly == n_layers - 1)
            with ExitStack() as lctx:
                ap_ = lctx.enter_context(tc.tile_pool(name=f"attn{ly}", bufs=1))
                apw = lctx.enter_context(tc.tile_pool(name=f"attnw{ly}", bufs=2))
                xh = ap_.tile([P, 8, 512], BF16, tag="xh")
                _layer_norm(nc, pools,
                            [h[:, r, :] for r in range(8)],
                            [xh[:, r, :] for r in range(8)],
                            lambda r: lnp["ln1g"][:, ly, r:r + 1],
                            lambda r: lnp["ln1b"][:, ly, r:r + 1])

                # ---- k, v (feed the merged pair AllGather), then q ----
                kTl = ap_.tile([P, 2, 512], BF16, tag="kTl")
                for ct in (8, 9):
                    wc = apw.tile([P, 8, P], BF16, name="wqkv_ct")
                    nc.sync.dma_start(wc[:], wqkv_d[ly, ct])
                    k_ps = ps_mm.tile([P, 512], F32, name="mm")
                    for kt in range(8):
                        nc.tensor.matmul(k_ps[:], wc[:, kt, :], xh[:, kt, :],
                                         start=(kt == 0), stop=(kt == 7))
                    nc.vector.tensor_copy(kTl[:, ct - 8, :], k_ps[:])
                nc.sync.dma_start(
                    kv_ag_in[KOFF:VOFF].rearrange("(c p t) -> p c t", p=P, c=2),
                    kTl[:])
                wv = apw.tile([P, 8, 256], BF16, name="wv")
                nc.sync.dma_start(wv[:], wv_d[ly])
                vloc = ap_.tile([P, 4, 256], BF16, tag="vloc")
                for tt in range(4):
                    v_ps = ps_mm.tile([P, 512], F32, name="mm")
                    for kt in range(8):
                        nc.tensor.matmul(v_ps[:, 0:256],
                                         xh[:, kt, tt * P:(tt + 1) * P],
                                         wv[:, kt, :],
                                         start=(kt == 0), stop=(kt == 7))
                    nc.vector.tensor_copy(vloc[:, tt, :], v_ps[:, 0:256])
                nc.sync.dma_start(
                    kv_ag_in[VOFF:QOFF].rearrange("(tt p c) -> p tt c",
                                                  p=P, tt=4),
                    vloc[:])
                if last:
                    # q rows for token m*64+63, channels 960:1024 (ct 7)
                    wc7 = apw.tile([P, 8, P], BF16, name="wqkv_ct")
                    nc.sync.dma_start(wc7[:], wqkv_d[ly, 7])
                    q8_ps = ps_mm.tile([P, 512], F32, name="mm")
                    for kt in range(8):
                        nc.tensor.matmul(q8_ps[0:64, 0:8],
                                         wc7[:, kt, 64:128],
                                         xh[:, kt, 63::64],
                                         start=(kt == 0), stop=(kt == 7))
                    q8 = ap_.tile([64, 8], BF16, tag="q8")
                    nc.vector.tensor_copy(q8[:], q8_ps[0:64, 0:8])
                    nc.sync.dma_start(
                        kv_ag_in[QOFF:HOFF].rearrange("(d m) -> d m", d=64),
                        q8[:])
                    hb = ap_.tile([P, 8], BF16, tag="hb")
                    nc.vector.tensor_copy(hb[:], h[:, :, 511])
                    nc.sync.dma_start(
                        kv_ag_in[HOFF:AGN].rearrange("(kt p) -> p kt", p=P),
                        hb[:])
                nc.gpsimd.collective_compute(
                    "AllGather", OP.bypass, replica_groups=PAIRS,
                    ins=[kv_ag_in[:]], outs=[kv_ag_out[:, :]])

                if not last:
                    qT = ap_.tile([P, 8, 512], BF16, tag="qT")
                    for ct in range(8):
                        wc = apw.tile([P, 8, P], BF16, name="wqkv_ct")
                        nc.sync.dma_start(wc[:], wqkv_d[ly, ct])
                        q_ps = ps_mm.tile([P, 512], F32, name="mm")
                        for kt in range(8):
                            nc.tensor.matmul(q_ps[:], wc[:, kt, :], xh[:, kt, :],
                                             start=(kt == 0), stop=(kt == 7))
                        nc.vector.tensor_copy(qT[:, ct, :], q_ps[:])

                # kT duplicated on both partition halves: [128, 4 h4, 1024 t]
                kTd = ap_.tile([P, 4, T], BF16, tag="kTd")
                for half in range(2):
                    src = kv_ag_out[half, KOFF:VOFF].rearrange(
                        "(h4 d t) -> d h4 t", h4=4, d=64)
                    nc.sync.dma_start(kTd[0:64, :, half * TL:(half + 1) * TL], src)
                    nc.sync.dma_start(kTd[64:128, :, half * TL:(half + 1) * TL], src)
                vst = ap_.tile([P, 8, 4, 65], BF16, tag="vst")
                nc.gpsimd.memset(vst[:, :, :, 64:65], 1.0)
                for hf in range(2):
                    for h4 in range(4):
                        nc.sync.dma_start(
                            vst[:, hf * 4:(hf + 1) * 4, h4, 0:64],
                            kv_ag_out[hf, VOFF:QOFF].rearrange(
                                "(tt p c) -> p tt c", p=P, tt=4)
                            [:, :, h4 * 64:(h4 + 1) * 64])

                if last:
                    _last_layer(nc, tc, pools, lctx, ap_, apw, ps_mm, ps_acc,
                                lnp, b3, h, kTd, vst, kv_ag_out,
                                wout3_d, wup3_d, wgate3_d, wdown3_d,
                                fin_ag_in, fin_ag_out, ly)
                    continue

                # ---- attention: 4 kv blocks x 2 units ----
                ost = [ap_.tile([P, 1024], BF16, tag=f"ost{r}", name=f"ost{r}")
                       for r in range(4)]
                for c in range(4):
                    o_ps = [[ps_acc.tile([P, 512], F32, name=f"acc{u * 2 + lh}")
                             for lh in range(2)] for u in range(2)]
                    for jt in range(8):
                        h4, tlt = jt // 2, jt % 2
                        tl0 = tlt * P
                        ta0 = 32 * tlt         # tri-skip: tau range [ta0, 64)
                        a_chunk = apw.tile([P, 2, 8, 2, 64], BF16, name="a_chunk")
                        for par in range(2):
                            b0 = par * 64
                            for hq in range(2):
                                s_ps = ps_mm.tile([P, 4, 2, 64], F32, name="mm")
                                # one matmul for all 4 heads of the hq group:
                                # same stationary k-tile, N=512/256
                                lhsT = kTd[b0:b0 + 64, h4,
                                           c * 256 + tl0: c * 256 + tl0 + P]
                                rhs = qT[b0:b0 + 64,
                                         hq * 4:(hq + 1) * 4, :].rearrange(
                                    "p h (blk tau) -> p h blk tau",
                                    tau=64)[:, :, c::4, ta0:64]
                                nc.tensor.matmul(s_ps[:, :, :, ta0:64],
                                                 lhsT, rhs,
                                                 start=True, stop=True)
                                nc.scalar.activation(
                                    a_chunk[:, par, hq * 4:(hq + 1) * 4, :,
                                            ta0:64],
                                    s_ps[:, :, :, ta0:64],
                                    AF.Exp, scale=0.125)
                        warm(6)
                        nc.vector.tensor_mul(
                            a_chunk[:, :, :, :, ta0:ta0 + 32],
                            a_chunk[:, :, :, :, ta0:ta0 + 32],
                            masks[h4][:])
                        tt8 = (c * 256 + tl0) // P
                        for u in range(2):
                            for lh in range(2):
                                if lh == 0 and tlt == 1:
                                    continue    # fully masked quarter
                                rhs = a_chunk[:, :, :, u, lh * 32:(lh + 1) * 32]
                                nc.tensor.matmul(
                                    o_ps[u][lh][0:65, :],
                                    vst[:, tt8, h4, :], rhs,
                                    start=(jt == 0),
                                    stop=(jt == 7 if lh else jt == 6))
                    # evacuate raw (frees PSUM fast); normalize afterwards
                    oraw = apw.tile([P, 4, 512], BF16, name="oraw")
                    for u in range(2):
                        for lh in range(2):
                            nc.vector.tensor_copy(oraw[0:65, u * 2 + lh, :],
                                                  o_ps[u][lh][0:65, :])
                    warm(10)
                    for u in range(2):
                        r = u * 2 + (c // 2)
                        for lh in range(2):
                            rcp = wk.tile([1, 512], F32, name="rcp")
                            nc.vector.reciprocal(rcp[:],
                                                 oraw[64:65, u * 2 + lh, :])
                            rcb = wk.tile([64, 512], F32, name="rcb")
                            nc.gpsimd.partition_broadcast(rcb[:], rcp[:])
                            nc.vector.tensor_mul(
                                ost[r][(c % 2) * 64:(c % 2) * 64 + 64,
                                       lh * 512:(lh + 1) * 512],
                                oraw[0:64, u * 2 + lh, :], rcb[:])

                # ---- out-projection + pair reduce-scatter ----
                for rout in range(8):
                    woc = apw.tile([P, 4, P], BF16, name="wocol")
                    nc.sync.dma_start(woc[:], wout_d[ly, rout])
                    for lh in range(2):
                        p_ps = ps_mm.tile([P, 512], F32, name="mm")
                        for kt in range(4):
                            rhs = ost[kt][:, lh * 512:(lh + 1) * 512].rearrange(
                                "p (par hidx tau) -> p tau hidx par",
                                par=2, hidx=8)
                            nc.tensor.matmul(p_ps[:], woc[:, kt, :], rhs,
                                             start=(kt == 0), stop=(kt == 3))
                        ap_sb = wk.tile([P, 512], BF16, name="ap_sb")
                        nc.vector.tensor_copy(ap_sb[:], p_ps[:])
                        nc.sync.dma_start(rs_in[lh, rout, :, :], ap_sb[:])
                nc.gpsimd.collective_compute(
                    "ReduceScatter", OP.add, replica_groups=PAIRS,
                    ins=[rs_in[:, :, :, :]], outs=[rs_out[:, :, :]])
                for r in range(8):
                    at = wk.tile([P, 512], BF16, name="at_sb")
                    nc.sync.dma_start(at[:], rs_out[r, :, :])
                    atf = wk.tile([P, 512], F32, name="atf_sb")
                    nc.vector.tensor_copy(atf[:], at[:])
                    nc.vector.tensor_add(h[:, r, :], h[:, r, :], atf[:])

            if last:
                continue
            # ---------------- FFN ----------------
            with ExitStack() as fctx:
                fp = fctx.enter_context(tc.tile_pool(name=f"ffn{ly}", bufs=1))
                fpw = fctx.enter_context(tc.tile_pool(name=f"ffnw{ly}", bufs=4))
                fps = fctx.enter_context(tc.tile_pool(name=f"ffns{ly}", bufs=2))
                x2 = fp.tile([P, 8, 512], BF16, tag="x2")
                _layer_norm(nc, pools,
                            [h[:, r, :] for r in range(8)],
                            [x2[:, r, :] for r in range(8)],
                            lambda r: lnp["ln2g"][:, ly, r:r + 1],
                            lambda r: lnp["ln2b"][:, ly, r:r + 1])
                hg = fp.tile([P, 32, 512], BF16, tag="hg")
                for ch in range(16):          # F chunks of 256
                    wu = fpw.tile([P, 8, 256], BF16, name="wup")
                    nc.sync.dma_start(wu[:], wup_d[ly, ch])
                    wg = fpw.tile([P, 8, 256], BF16, name="wgate")
                    nc.sync.dma_start(wg[:], wgate_d[ly, ch])
                    for fi in range(2):       # F-tiles of 128 in chunk
                        ft = ch * 2 + fi
                        u_ps = ps_mm.tile([P, 512], F32, name="mm")
                        for kt in range(8):
                            nc.tensor.matmul(u_ps[:], wu[:, kt, fi * P:(fi + 1) * P],
                                             x2[:, kt, :],
                                             start=(kt == 0), stop=(kt == 7))
                        g_ps = ps_mm.tile([P, 512], F32, name="mm")
                        for kt in range(8):
                            nc.tensor.matmul(g_ps[:], wg[:, kt, fi * P:(fi + 1) * P],
                                             x2[:, kt, :],
                                             start=(kt == 0), stop=(kt == 7))
                        u_sb = fps.tile([P, 512], BF16, name="u_sb")
                        nc.scalar.activation(u_sb[:], u_ps[:], AF.Identity,
                                             bias=lnp["bup"][:, ly, ft:ft + 1])
                        g_sb = fps.tile([P, 512], BF16, name="g_sb")
                        nc.scalar.activation(g_sb[:], g_ps[:], AF.Gelu_apprx_tanh,
                                             bias=lnp["bgate"][:, ly, ft:ft + 1])
                        nc.vector.tensor_mul(hg[:, ft, :], u_sb[:], g_sb[:])
                # down: 2 groups of 4 out-tiles, Wdown streamed per group
                for grp in range(2):
                    d_ps = [ps_acc.tile([P, 512], F32, name=f"acc{i}")
                            for i in range(4)]
                    for kt in range(32):
                        wd = fpw.tile([P, 1024], BF16, name="wdown")
                        nc.sync.dma_start(wd[:],
                                          wdown_d[ly, kt * P:(kt + 1) * P, :])
                        for i in range(4):
                            rout = grp * 4 + i
                            nc.tensor.matmul(d_ps[i][:],
                                             wd[:, rout * P:(rout + 1) * P],
                                             hg[:, kt, :],
                                             start=(kt == 0), stop=(kt == 31))
                    for i in range(4):
                        rout = grp * 4 + i
                        dn = fps.tile([P, 512], F32, name="dn_sb")
                        nc.scalar.activation(dn[:], d_ps[i][:], AF.Identity,
                                             bias=lnp["bdown"][:, ly, rout:rout + 1])
                        nc.vector.tensor_add(h[:, rout, :], h[:, rout, :], dn[:])

        # -------- tail: assemble h_fin for 4 batches, final LN, LM head ------
        with ExitStack() as tctx:
            tp = tctx.enter_context(tc.tile_pool(name="tail", bufs=6))
            he = tp.tile([P, 8, 4, 2], F32, name="he")
            for bb in range(4):
                for s in range(2):
                    nc.gpsimd.dma_start(
                        he[:, :, bb, s],
                        fin_ag_out[2 * bb + s].rearrange("(r p) -> p r", p=P))
            hfin4 = tp.tile([P, 8, 4], F32R, name="hfin4")
            nc.vector.tensor_add(hfin4[:], he[:, :, :, 0], he[:, :, :, 1])
            hall = tp.tile([P, 8, 4], BF16, name="hall")
            _vec_ln(nc, pools, tp, hfin4,
                    lambda r: hall[:, r, :],
                    lambda r: fln[:, 0, r:r + 1],
                    lambda r: fln[:, 1, r:r + 1], 4)
            for nt in range(8):
                n0, n1 = nt * 500, (nt + 1) * 500
                l_ps = ps_mm.tile([P, 512], F32, name="mm")
                for kt in range(8):
                    if n1 <= 2000:
                        wl_ap = wlmA[:, kt, n0:n1]
                    else:
                        wl = tp.tile([P, 500], BF16, name="wlm")
                        nc.sync.dma_start(wl[:], wlm_d[:, kt, n0:n1])
                        wl_ap = wl[:]
                    nc.tensor.matmul(l_ps[0:4, 0:500], hall[:, kt, :], wl_ap,
                                     start=(kt == 0), stop=(kt == 7))
                bl = tp.tile([1, 500], F32, name="blm")
                nc.sync.dma_start(bl[:], blm_d[n0:n1])
                blb = tp.tile([4, 500], F32, name="blb")
                nc.gpsimd.partition_broadcast(blb[:], bl[:])
                lo = tp.tile([4, 512], F32, name="lo")
                nc.vector.tensor_add(lo[:, 0:500], l_ps[0:4, 0:500], blb[:])
                nc.sync.dma_start(logits_d[:, n0:n1], lo[:, 0:500])

    nc.compile()
    return nc


def _last_layer(nc, tc, pools, lctx, ap_, apw, ps_mm, ps_acc, lnp, b3, h,
                kTd, vst, kv_ag_out, wout3_d, wup3_d, wgate3_d, wdown3_d,
                fin_ag_in, fin_ag_out, ly):
    """Layer 3: only token 1023 survives -> 1-token attention + out-proj
    (duplicated on both pair cores) and a pair-F-sharded 1-token FFN."""
    wk = pools["wk"]
    # q rows for all 16 units: [64 d, 16 m]
    qa = ap_.tile([64, 16], BF16, tag="qa")
    for s in range(2):
        nc.sync.dma_start(
            qa[0:64, s * 8:(s + 1) * 8],
            kv_ag_out[s, QOFF:HOFF].rearrange("(d m) -> d m", d=64))
    h1023b = ap_.tile([P, 8], BF16, tag="h1023b")
    nc.sync.dma_start(h1023b[:],
                      kv_ag_out[1, HOFF:AGN].rearrange("(kt p) -> p kt", p=P))

    # 1-token attention, no mask (l=1023 attends everything)
    of = ap_.tile([64, 4, 4], BF16, tag="of")      # [d, c, mi]
    for c in range(4):
        s_ps = ps_mm.tile([P, 512], F32, name="mm")
        for th in range(2):
            for h4q in range(4):
                col = (th * 4 + h4q) * 4
                lhsT = kTd[0:64, h4q,
                           c * 256 + th * P: c * 256 + (th + 1) * P]
                nc.tensor.matmul(s_ps[:, col:col + 4], lhsT, qa[0:64, c::4],
                                 start=True, stop=True)
        a1 = apw.tile([P, 32], BF16, name="a1")
        nc.scalar.activation(a1[:], s_ps[:, 0:32], AF.Exp, scale=0.125)
        o1_ps = ps_acc.tile([P, 512], F32, name="acc0")
        for th in range(2):
            for h4q in range(4):
                col = (th * 4 + h4q) * 4
                nc.tensor.matmul(o1_ps[0:65, 0:4],
                                 vst[:, c * 2 + th, h4q, :],
                                 a1[:, col:col + 4],
                                 start=(th == 0 and h4q == 0),
                                 stop=(th == 1 and h4q == 3))
        rcp1 = wk.tile([1, 512], F32, name="rcp")
        nc.vector.reciprocal(rcp1[0:1, 0:4], o1_ps[64:65, 0:4])
        rcb1 = wk.tile([64, 512], F32, name="rcb")
        nc.gpsimd.partition_broadcast(rcb1[0:64, 0:4], rcp1[0:1, 0:4])
        nc.vector.tensor_mul(of[:, c, :], o1_ps[0:64, 0:4], rcb1[0:64, 0:4])

    # assemble o_final^T: chan = m*64 + d -> ofT[(m%2)*64 + d, m//2]
    ofT = ap_.tile([P, 8], BF16, tag="ofT")
    for c in range(4):
        for mi in range(4):
            m = mi * 4 + c
            p0 = (m % 2) * 64
            nc.vector.tensor_copy(ofT[p0:p0 + 64, m // 2:m // 2 + 1],
                                  of[:, c, mi:mi + 1])

    # 1-token out-projection (full contraction, duplicated on both cores)
    ya = ap_.tile([P, 8], F32, tag="ya")
    for r_out in range(8):
        woc3 = apw.tile([P, 8, P], BF16, name="woc3")
        nc.sync.dma_start(woc3[:], wout3_d[r_out])
        y_ps = ps_mm.tile([P, 512], F32, name="mm")
        for kt in range(8):
            nc.tensor.matmul(y_ps[:, 0:1], woc3[:, kt, :], ofT[:, kt:kt + 1],
                             start=(kt == 0), stop=(kt == 7))
        nc.vector.tensor_copy(ya[:, r_out:r_out + 1], y_ps[:, 0:1])

    h23 = ap_.tile([P, 8], F32, tag="h23")
    h1023f = ap_.tile([P, 8], F32, tag="h1023f")
    nc.vector.tensor_copy(h1023f[:], h1023b[:])
    nc.vector.tensor_add(h23[:], h1023f[:], ya[:])

    # LN2 on the single token (duplicate the column for even-N f32r matmuls)
    ht2 = ap_.tile([P, 8, 2], F32R, tag="ht2")
    nc.vector.tensor_copy(ht2[:, :, 0], h23[:])
    nc.vector.tensor_copy(ht2[:, :, 1], h23[:])
    x2t = ap_.tile([P, 8, 2], BF16, tag="x2t")
    _vec_ln(nc, pools, apw, ht2,
            lambda r: x2t[:, r, :],
            lambda r: lnp["ln2g"][:, ly, r:r + 1],
            lambda r: lnp["ln2b"][:, ly, r:r + 1], 2)

    # FFN on my F-half (2048 features), feature-major
    hg3 = ap_.tile([P, 16], BF16, tag="hg3")
    for fc in range(16):
        wu3 = apw.tile([P, 8, P], BF16, name="wu3")
        nc.sync.dma_start(wu3[:], wup3_d[fc])
        wg3 = apw.tile([P, 8, P], BF16, name="wg3")
        nc.sync.dma_start(wg3[:], wgate3_d[fc])
        u_ps = ps_mm.tile([P, 512], F32, name="mm")
        for kt in range(8):
            nc.tensor.matmul(u_ps[:, 0:1], wu3[:, kt, :], x2t[:, kt, 0:1],
                             start=(kt == 0), stop=(kt == 7))
        g_ps = ps_mm.tile([P, 512], F32, name="mm")
        for kt in range(8):
            nc.tensor.matmul(g_ps[:, 0:1], wg3[:, kt, :], x2t[:, kt, 0:1],
                             start=(kt == 0), stop=(kt == 7))
        u_sb = wk.tile([P, 1], BF16, name="u3_sb")
        nc.scalar.activation(u_sb[:], u_ps[:, 0:1], AF.Identity,
                             bias=b3[:, 0, fc:fc + 1])
        g_sb = wk.tile([P, 1], BF16, name="g3_sb")
        nc.scalar.activation(g_sb[:], g_ps[:, 0:1], AF.Gelu_apprx_tanh,
                             bias=b3[:, 1, fc:fc + 1])
        nc.vector.tensor_mul(hg3[:, fc:fc + 1], u_sb[:], g_sb[:])
    y3_ps = ps_acc.tile([P, 512], F32, name="acc1")
    for dc in range(8):
        wd3 = apw.tile([P, 16, P], BF16, name="wd3")
        nc.sync.dma_start(wd3[:], wdown3_d[dc])
        for fk in range(16):
            nc.tensor.matmul(y3_ps[:, dc:dc + 1], wd3[:, fk, :],
                             hg3[:, fk:fk + 1],
                             start=(fk == 0), stop=(fk == 15))
    # s_fin = 0.5*(h23 + bdown) + y3_partial ; pair sums in the final AG
    tmp = ap_.tile([P, 8], F32, tag="sfin_tmp")
    nc.vector.tensor_add(tmp[:], h23[:], lnp["bdown"][:, ly, :])
    y3_sb = ap_.tile([P, 8], F32, tag="y3_sb")
    nc.vector.tensor_copy(y3_sb[:], y3_ps[:, 0:8])
    s_fin = ap_.tile([P, 8], F32, tag="s_fin")
    nc.vector.scalar_tensor_tensor(s_fin[:], tmp[:], 0.5, y3_sb[:],
                                   op0=OP.mult, op1=OP.add)
    nc.sync.dma_start(fin_ag_in.ap().rearrange("(r p) -> p r", p=P), s_fin[:])
    nc.gpsimd.collective_compute(
        "AllGather", OP.bypass, replica_groups=ALL8,
        ins=[fin_ag_in[:]], outs=[fin_ag_out[:, :]])


def _pe_table(t, d):
    pos = np.arange(t, dtype=np.float32)[:, None]
    freq = np.exp(-(np.arange(0, d, 2, dtype=np.float32) / d) * np.log(10000.0))
    ang = pos * freq[None, :]
    pe = np.zeros((t, d), dtype=np.float32)
    pe[:, 0::2] = np.sin(ang)
    pe[:, 1::2] = np.cos(ang)
    return pe


def _prepack(Wqkv, Wup, Wgate, Wdown, Wlm, nl):
    """Host-side bf16 prepack into per-tile DMA layouts."""
    bf = lambda x: np.ascontiguousarray(x).astype(NPBF16)
    wq = Wqkv[:, :, :1280].reshape(nl, 8, P, 10, P)      # [l, kt, p, ct, c]
    wqkvP = bf(wq.transpose(0, 3, 2, 1, 4))
    wvP = bf(Wqkv[:, :, 1280:1536].reshape(nl, 8, P, 256).transpose(0, 2, 1, 3))
    wupP = bf(Wup.reshape(nl, 8, P, 16, 256).transpose(0, 3, 2, 1, 4))
    wgateP = bf(Wgate.reshape(nl, 8, P, 16, 256).transpose(0, 3, 2, 1, 4))
    wdownP = bf(Wdown)
    wlmP = bf(Wlm.reshape(8, P, V).transpose(1, 0, 2))   # [p, kt, V]
    return wqkvP, wvP, wupP, wgateP, wdownP, wlmP


def kernel(idx, emb, Wqkv, Wout, ln1_g, ln1_b, ln2_g, ln2_b, Wup, bup,
           Wgate, bgate, Wdown, bdown, fln_g, fln_b, Wlm, blm, _trace=False):
    f32 = lambda x: np.ascontiguousarray(np.asarray(x, dtype=np.float32))
    bf = lambda x: np.ascontiguousarray(np.asarray(x)).astype(NPBF16)
    idx = np.asarray(idx)
    emb = f32(emb)
    Wqkv, Wout, Wup, Wgate, Wdown, Wlm = map(
        lambda x: np.asarray(x, dtype=np.float32),
        (Wqkv, Wout, Wup, Wgate, Wdown, Wlm))
    blm_f = f32(blm)
    bup_f, bgate_f = f32(bup), f32(bgate)

    nl = int(Wqkv.shape[0])
    if ("nc", nl) not in _CACHE:
        _CACHE[("nc", nl)] = build_kernel(nl)
    nc = _CACHE[("nc", nl)]

    wqkvP, wvP, wupP, wgateP, wdownP, wlmP = _prepack(
        Wqkv, Wup, Wgate, Wdown, Wlm, nl)
    wout3P = bf(Wout[nl - 1].reshape(8, P, 8, P).transpose(2, 1, 0, 3))

    pe = _pe_table(T, D)
    h0 = emb[np.asarray(idx)] * np.float32(np.sqrt(D)) + pe[None]  # [B, T, D]

    in_maps = []
    for core in range(8):
        b, half = core // 2, core % 2
        t0 = half * TL
        wo = Wout[:, t0:t0 + TL, :].reshape(nl, 4, P, 8, P)
        woutP = bf(wo.transpose(0, 3, 2, 1, 4))
        fsl = slice(half * 16, (half + 1) * 16)
        wup3P = bf(Wup[nl - 1].reshape(8, P, 32, P)[:, :, fsl, :]
                   .transpose(2, 1, 0, 3))
        wgate3P = bf(Wgate[nl - 1].reshape(8, P, 32, P)[:, :, fsl, :]
                     .transpose(2, 1, 0, 3))
        wdown3P = bf(Wdown[nl - 1].reshape(32, P, 8, P)[fsl, :, :, :]
                     .transpose(2, 1, 0, 3))
        bup3P = np.ascontiguousarray(
            bup_f[nl - 1].reshape(32, P)[fsl].T)
        bgate3P = np.ascontiguousarray(
            bgate_f[nl - 1].reshape(32, P)[fsl].T)
        in_maps.append({
            "h0t": np.ascontiguousarray(h0[b, t0:t0 + TL].T),
            "wqkv": wqkvP, "wv": wvP,
            "wout": woutP, "wout3": wout3P,
            "wup": wupP, "wgate": wgateP, "wdown": wdownP,
            "wup3": wup3P, "wgate3": wgate3P, "wdown3": wdown3P,
            "bup3": bup3P, "bgate3": bgate3P,
            "ln1g": f32(ln1_g), "ln1b": f32(ln1_b),
            "ln2g": f32(ln2_g), "ln2b": f32(ln2_b),
            "bup": bup_f, "bgate": bgate_f, "bdown": f32(bdown),
            "flng": f32(fln_g), "flnb": f32(fln_b),
            "wlm": np.ascontiguousarray(wlmP[:, :, core * VC:(core + 1) * VC]),
            "blm": np.ascontiguousarray(blm_f[core * VC:(core + 1) * VC]),
        })
    res = run_bass_kernel_spmd(nc, in_maps, core_ids=list(range(8)),
                               trace=_trace)
    logits = np.zeros((B, 1, V), dtype=np.float32)
    for core in range(8):
        logits[:, 0, core * VC:(core + 1) * VC] = res.results[core]["logits"]
    if _trace:
        return logits, res
    return logits


# revision 47
# speedup vs baseline: 1.0948x; 1.0948x over previous
"""Trainium2 Bass kernel for nn_DecoderModel_54795192762653.

4-layer decoder, B=4, T=1024, D=1024, H=16, K=4 kv-heads, HD=64, F=4096,
V=32000. 8 NeuronCores: pair (2b, 2b+1) handles batch b; within a pair,
core A owns tokens 0..511 and core B owns 512..1023.

v3 changes vs v2:
- single merged pair-AllGather per layer (k + v + [layer-3: q-rows + h_1023])
- ReduceScatter split in two (rout 0-3 / 4-7) to overlap wire with matmuls
- attention o-accumulators evacuated raw to SBUF (fast PSUM free), softmax
  normalization deferred off the critical path
- full-shape contiguous causal masks (8 variants), DVE multiply
- last layer specialized: only token 1023 survives the block, so layer 3
  computes k/v for all tokens plus a 16-row q slice, a 1-token attention,
  a 1-token out-projection, and a pair-F-sharded 1-token FFN; partial sums
  meet in the final 8-core AllGather
- LM head weights half-preloaded into SBUF at kernel start

Attention uses the reference's "scrambled" reshape semantics: unit m
(m = g*4 + kv) reads q rows m*64..(m+1)*64 (all channels) viewed as
(1024 l x 64 d); k/v block c = m % 4 rows c*256..(c+1)*256 viewed as
(1024 j x 64 d). Scores are computed transposed (j on partitions,
j = 4*(token offset in c-block) + h4), l = 16*tau + 2*hidx + par.
Softmax denominator comes from a ones-column appended to V (M=65 matmul).
"""
import sys

sys.path.insert(0, "/opt/trn_rl_repo")

import numpy as np
import ml_dtypes
from contextlib import ExitStack

import concourse.bass as bass
import concourse.tile as tile
from concourse import bacc, mybir
from concourse.bass_utils import run_bass_kernel_spmd

P = 128
F32 = mybir.dt.float32
F32R = mybir.dt.float32r
BF16 = mybir.dt.bfloat16
U32 = mybir.dt.uint32
AF = mybir.ActivationFunctionType
OP = mybir.AluOpType
NPBF16 = ml_dtypes.bfloat16

D, H, KV, F, L, V, T, B = 1024, 16, 4, 4096, 4, 32000, 1024, 4
HD = D // H
TL = T // 2          # 512 tokens per core
VC = V // 8          # 4000 vocab per core
EPS = 1e-5
PAIRS = [[0, 1], [2, 3], [4, 5], [6, 7]]
ALL8 = [list(range(8))]

# merged AG buffer regions (bf16 elements)
KOFF, VOFF, QOFF, HOFF, AGN = 0, 131072, 262144, 262656, 263680

_CACHE = {}


def _layer_norm(nc, pools, h_tiles, out_tiles, g_ap, b_ap):
    """Feature-major layernorm over D=1024 (8 partition tiles x 512 tokens)."""
    wk, ps_mm, ones_col = pools["wk"], pools["ps_mm"], pools["ones_col"]
    s1 = ps_mm.tile([P, 512], F32, name="mm")
    s2 = ps_mm.tile([P, 512], F32, name="mm")
    for r in range(8):
        nc.tensor.matmul(s1[0:1, :], ones_col[:, 0:1], h_tiles[r],
                         start=(r == 0), stop=(r == 7))
    for r in range(8):
        sq = wk.tile([P, 512], F32R, name="ln_sq")
        nc.scalar.activation(sq[:], h_tiles[r], AF.Square)
        nc.tensor.matmul(s2[0:1, :], ones_col[:, 0:1], sq[:],
                         start=(r == 0), stop=(r == 7))
    mu = wk.tile([1, 512], F32, name="ln_mu")
    nc.scalar.mul(mu[:], s1[0:1, :], 1.0 / D)
    e2 = wk.tile([1, 512], F32, name="ln_e2")
    nc.scalar.mul(e2[:], s2[0:1, :], 1.0 / D)
    musq = wk.tile([1, 512], F32, name="ln_musq")
    nc.scalar.activation(musq[:], mu[:], AF.Square)
    var = wk.tile([1, 512], F32, name="ln_var")
    nc.vector.tensor_sub(var[:], e2[:], musq[:])
    sd = wk.tile([1, 512], F32, name="ln_sd")
    nc.scalar.activation(sd[:], var[:], AF.Sqrt, bias=pools["eps"][0:1, :])
    rv = wk.tile([1, 512], F32, name="ln_rv")
    nc.vector.reciprocal(rv[:], sd[:])
    cv = wk.tile([1, 512], F32, name="ln_cv")
    nc.vector.scalar_tensor_tensor(cv[:], mu[:], -1.0, rv[:],
                                   op0=OP.mult, op1=OP.mult)
    rb = wk.tile([P, 512], F32, name="ln_rb")
    nc.gpsimd.partition_broadcast(rb[:], rv[:])
    cb = wk.tile([P, 512], F32, name="ln_cb")
    nc.gpsimd.partition_broadcast(cb[:], cv[:])
    for r in range(8):
        t1 = wk.tile([P, 512], F32, name="ln_t1")
        nc.vector.tensor_mul(t1[:], h_tiles[r], rb[:])
        nc.vector.tensor_add(t1[:], t1[:], cb[:])
        nc.scalar.activation(out_tiles[r], t1[:], AF.Identity,
                             bias=b_ap(r), scale=g_ap(r))


def _vec_ln(nc, pools, pool, src2, out_fn, g_ap, b_ap, nb):
    """Feature-major layernorm of nb token columns. src2: [P, 8, nb] F32R
    (nb even); writes out via out_fn(r) -> [P, nb] APs (may be bf16)."""
    ps_mm, ones_col, eps_t = pools["ps_mm"], pools["ones_col"], pools["eps"]
    s1 = ps_mm.tile([P, 512], F32, name="mm")
    s2 = ps_mm.tile([P, 512], F32, name="mm")
    for r in range(8):
        nc.tensor.matmul(s1[0:1, 0:nb], ones_col[:, 0:1], src2[:, r, :],
                         start=(r == 0), stop=(r == 7))
    for r in range(8):
        sqf = pool.tile([P, 8], F32R, name="vln_sq")
        nc.scalar.activation(sqf[:, 0:nb], src2[:, r, :], AF.Square)
        nc.tensor.matmul(s2[0:1, 0:nb], ones_col[:, 0:1], sqf[:, 0:nb],
                         start=(r == 0), stop=(r == 7))
    mu = pool.tile([1, 8], F32, name="vln_mu")
    nc.scalar.mul(mu[0:1, 0:nb], s1[0:1, 0:nb], 1.0 / D)
    e2 = pool.tile([1, 8], F32, name="vln_e2")
    nc.scalar.mul(e2[0:1, 0:nb], s2[0:1, 0:nb], 1.0 / D)
    musq = pool.tile([1, 8], F32, name="vln_musq")
    nc.scalar.activation(musq[0:1, 0:nb], mu[0:1, 0:nb], AF.Square)
    var = pool.tile([1, 8], F32, name="vln_var")
    nc.vector.tensor_sub(var[0:1, 0:nb], e2[0:1, 0:nb], musq[0:1, 0:nb])
    sd = pool.tile([1, 8], F32, name="vln_sd")
    nc.scalar.activation(sd[0:1, 0:nb], var[0:1, 0:nb], AF.Sqrt,
                         bias=eps_t[0:1, :])
    rv = pool.tile([1, 8], F32, name="vln_rv")
    nc.vector.reciprocal(rv[0:1, 0:nb], sd[0:1, 0:nb])
    cv = pool.tile([1, 8], F32, name="vln_cv")
    nc.vector.scalar_tensor_tensor(cv[0:1, 0:nb], mu[0:1, 0:nb], -1.0,
                                   rv[0:1, 0:nb], op0=OP.mult, op1=OP.mult)
    rb = pool.tile([P, 8], F32, name="vln_rb")
    nc.gpsimd.partition_broadcast(rb[:, 0:nb], rv[0:1, 0:nb])
    cb = pool.tile([P, 8], F32, name="vln_cb")
    nc.gpsimd.partition_broadcast(cb[:, 0:nb], cv[0:1, 0:nb])
    for r in range(8):
        t1 = pool.tile([P, 8], F32, name="vln_t1")
        nc.vector.tensor_mul(t1[:, 0:nb], src2[:, r, :], rb[:, 0:nb])
        nc.vector.tensor_add(t1[:, 0:nb], t1[:, 0:nb], cb[:, 0:nb])
        nc.scalar.activation(out_fn(r), t1[:, 0:nb], AF.Identity,
                             bias=b_ap(r), scale=g_ap(r))


def build_kernel(n_layers=L):
    nc = bacc.Bacc("TRN2", target_bir_lowering=False, debug=False, num_devices=8)

    # ---------------- I/O ----------------
    h0t_d = nc.dram_tensor("h0t", [D, TL], F32R, kind="ExternalInput")
    wqkv_d = nc.dram_tensor("wqkv", [n_layers, 10, P, 8, P], BF16,
                            kind="ExternalInput")
    wv_d = nc.dram_tensor("wv", [n_layers, P, 8, 256], BF16,
                          kind="ExternalInput")
    wout_d = nc.dram_tensor("wout", [n_layers, 8, P, 4, P], BF16,
                            kind="ExternalInput")
    wout3_d = nc.dram_tensor("wout3", [8, P, 8, P], BF16, kind="ExternalInput")
    wup_d = nc.dram_tensor("wup", [n_layers, 16, P, 8, 256], BF16,
                           kind="ExternalInput")
    wgate_d = nc.dram_tensor("wgate", [n_layers, 16, P, 8, 256], BF16,
                             kind="ExternalInput")
    wdown_d = nc.dram_tensor("wdown", [n_layers, F, D], BF16,
                             kind="ExternalInput")
    wup3_d = nc.dram_tensor("wup3", [16, P, 8, P], BF16, kind="ExternalInput")
    wgate3_d = nc.dram_tensor("wgate3", [16, P, 8, P], BF16,
                              kind="ExternalInput")
    wdown3_d = nc.dram_tensor("wdown3", [8, P, 16, P], BF16,
                              kind="ExternalInput")
    bup3_d = nc.dram_tensor("bup3", [P, 16], F32, kind="ExternalInput")
    bgate3_d = nc.dram_tensor("bgate3", [P, 16], F32, kind="ExternalInput")
    ln1g_d = nc.dram_tensor("ln1g", [n_layers, D], F32, kind="ExternalInput")
    ln1b_d = nc.dram_tensor("ln1b", [n_layers, D], F32, kind="ExternalInput")
    ln2g_d = nc.dram_tensor("ln2g", [n_layers, D], F32, kind="ExternalInput")
    ln2b_d = nc.dram_tensor("ln2b", [n_layers, D], F32, kind="ExternalInput")
    bup_d = nc.dram_tensor("bup", [n_layers, F], F32, kind="ExternalInput")
    bgate_d = nc.dram_tensor("bgate", [n_layers, F], F32, kind="ExternalInput")
    bdown_d = nc.dram_tensor("bdown", [n_layers, D], F32, kind="ExternalInput")
    flng_d = nc.dram_tensor("flng", [D], F32, kind="ExternalInput")
    flnb_d = nc.dram_tensor("flnb", [D], F32, kind="ExternalInput")
    wlm_d = nc.dram_tensor("wlm", [P, 8, VC], BF16, kind="ExternalInput")
    blm_d = nc.dram_tensor("blm", [VC], F32, kind="ExternalInput")
    logits_d = nc.dram_tensor("logits", [B, VC], F32, kind="ExternalOutput")

    # collective bounce buffers (internal DRAM)
    kv_ag_in = nc.dram_tensor("kv_ag_in", [AGN], BF16)
    kv_ag_out = nc.dram_tensor("kv_ag_out", [2, AGN], BF16)
    rs_in = nc.dram_tensor("rs_in", [2, 8, P, TL], BF16)      # [half, rout]
    rs_out = nc.dram_tensor("rs_out", [8, P, TL], BF16)
    # tiny dummy collectives to warm up ncfw for both replica-group shapes
    wu_ag_in = nc.dram_tensor("wu_ag_in", [64], BF16)
    wu_ag_out = nc.dram_tensor("wu_ag_out", [2, 64], BF16)
    wu8_ag_in = nc.dram_tensor("wu8_ag_in", [64], BF16)
    wu8_ag_out = nc.dram_tensor("wu8_ag_out", [8, 64], BF16)
    fin_ag_in = nc.dram_tensor("fin_ag_in", [D], F32)
    fin_ag_out = nc.dram_tensor("fin_ag_out", [8, D], F32, addr_space="Shared")

    with tile.TileContext(nc) as tc, ExitStack() as ctx:
        pers = ctx.enter_context(tc.tile_pool(name="pers", bufs=1))
        wk = ctx.enter_context(tc.tile_pool(name="wk", bufs=2))
        ps_mm = ctx.enter_context(tc.tile_pool(name="ps_mm", bufs=3, space="PSUM"))
        ps_acc = ctx.enter_context(tc.tile_pool(name="ps_acc", bufs=1, space="PSUM"))
        pools = {"wk": wk, "ps_mm": ps_mm}

        # initial residual first in the DMA queue: LN1 of layer 0 needs it
        h = pers.tile([P, 8, 512], F32R, tag="h")      # residual stream h^T
        nc.sync.dma_start(h[:], h0t_d.ap().rearrange("(kt p) t -> p kt t", p=P))

        # warm up ncfw for both replica-group shapes while startup DMAs fly
        with tc.high_priority():
            nc.gpsimd.collective_compute(
                "AllGather", OP.bypass, replica_groups=PAIRS,
                ins=[wu_ag_in[:]], outs=[wu_ag_out[:, :]])
            nc.gpsimd.collective_compute(
                "AllGather", OP.bypass, replica_groups=ALL8,
                ins=[wu8_ag_in[:]], outs=[wu8_ag_out[:, :]])

        # ---------------- constants ----------------
        ones_col = pers.tile([P, 1], F32R, tag="ones_col")
        nc.gpsimd.memset(ones_col[:].bitcast(F32), 1.0)
        pools["ones_col"] = ones_col
        eps_t = pers.tile([P, 1], F32, tag="eps")
        nc.gpsimd.memset(eps_t[:], EPS)
        pools["eps"] = eps_t

        # causal masks: keep iff l - j >= 0 with
        # l = 16*tau + 2*hidx + par, j = 512*tlt + 4*p + h4
        masks = []
        with ExitStack() as mctx:
            mpool = mctx.enter_context(tc.tile_pool(name="maskinit", bufs=2))
            for h4 in range(4):
                mf = mpool.tile([P, 2, 8, 2, 32], F32, name="maskf")
                nc.gpsimd.memset(mf[:], 1.0)
                nc.gpsimd.affine_select(
                    out=mf[:], in_=mf[:],
                    pattern=[[1, 2], [2, 8], [0, 2], [16, 32]],
                    channel_multiplier=-4, base=-h4,
                    compare_op=OP.is_ge, fill=0.0)
                mb = pers.tile([P, 2, 8, 2, 32], BF16, tag=f"maskb{h4}")
                nc.vector.tensor_copy(mb[:], mf[:])
                masks.append(mb)

        # LM head: preload first quarter (nt 0-1) into SBUF
        wlmA = pers.tile([P, 8, 2000], BF16, tag="wlmA")
        for kt in range(8):
            nc.sync.dma_start(wlmA[:, kt, :], wlm_d[:, kt, 0:2000])

        # ---------------- per-layer params (small, load all) ----------------
        lnp = {}
        for name, dram, nt in [("ln1g", ln1g_d, 8), ("ln1b", ln1b_d, 8),
                               ("ln2g", ln2g_d, 8), ("ln2b", ln2b_d, 8),
                               ("bup", bup_d, 32), ("bgate", bgate_d, 32),
                               ("bdown", bdown_d, 8)]:
            t = pers.tile([P, n_layers, nt], F32, tag=f"p_{name}")
            nc.sync.dma_start(t[:], dram.ap().rearrange("l (t p) -> p l t", p=P))
            lnp[name] = t
        fln = pers.tile([P, 2, 8], F32, tag="p_fln")
        nc.sync.dma_start(fln[:, 0], flng_d.ap().rearrange("(t p) -> p t", p=P))
        nc.sync.dma_start(fln[:, 1], flnb_d.ap().rearrange("(t p) -> p t", p=P))
        b3 = pers.tile([P, 2, 16], F32, tag="p_b3")
        nc.sync.dma_start(b3[:, 0], bup3_d[:, :])
        nc.sync.dma_start(b3[:, 1], bgate3_d[:, :])

        # ---------------- layers 0..n-2 (full) ----------------
        for ly in range(n_layers):
            last = (ly == n_layers - 1)
            with ExitStack() as lctx:
                ap_ = lctx.enter_context(tc.tile_pool(name=f"attn{ly}", bufs=1))
                apw = lctx.enter_context(tc.tile_pool(name=f"attnw{ly}", bufs=2))
                xh = ap_.tile([P, 8, 512], BF16, tag="xh")
                _layer_norm(nc, pools,
                            [h[:, r, :] for r in range(8)],
                            [xh[:, r, :] for r in range(8)],
                            lambda r: lnp["ln1g"][:, ly, r:r + 1],
                            lambda r: lnp["ln1b"][:, ly, r:r + 1])

                # ---- k, v (feed the merged pair AllGather), then q ----
                kTl = ap_.tile([P, 2, 512], BF16, tag="kTl")
                for ct in (8, 9):
                    wc = apw.tile([P, 8, P], BF16, name="wqkv_ct")
                    nc.sync.dma_start(wc[:], wqkv_d[ly, ct])
                    k_ps = ps_mm.tile([P, 512], F32, name="mm")
                    for kt in range(8):
                        nc.tensor.matmul(k_ps[:], wc[:, kt, :], xh[:, kt, :],
                                         start=(kt == 0), stop=(kt == 7))
                    nc.vector.tensor_copy(kTl[:, ct - 8, :], k_ps[:])
                nc.sync.dma_start(
                    kv_ag_in[KOFF:VOFF].rearrange("(c p t) -> p c t", p=P, c=2),
                    kTl[:])
                wv = apw.tile([P, 8, 256], BF16, name="wv")
                nc.sync.dma_start(wv[:], wv_d[ly])
                vloc = ap_.tile([P, 4, 256], BF16, tag="vloc")
                for tt in range(4):
                    v_ps = ps_mm.tile([P, 512], F32, name="mm")
                    for kt in range(8):
                        nc.tensor.matmul(v_ps[:, 0:256],
                                         xh[:, kt, tt * P:(tt + 1) * P],
                                         wv[:, kt, :],
                                         start=(kt == 0), stop=(kt == 7))
                    nc.vector.tensor_copy(vloc[:, tt, :], v_ps[:, 0:256])
                nc.sync.dma_start(
                    kv_ag_in[VOFF:QOFF].rearrange("(tt p c) -> p tt c",
                                                  p=P, tt=4),
                    vloc[:])
                if last:
                    # q rows for token m*64+63, channels 960:1024 (ct 7)
                    wc7 = apw.tile([P, 8, P], BF16, name="wqkv_ct")
                    nc.sync.dma_start(wc7[:], wqkv_d[ly, 7])
                    q8_ps = ps_mm.tile([P, 512], F32, name="mm")
                    for kt in range(8):
                        nc.tensor.matmul(q8_ps[0:64, 0:8],
                                         wc7[:, kt, 64:128],
                                         xh[:, kt, 63::64],
                                         start=(kt == 0), stop=(kt == 7))
                    q8 = ap_.tile([64, 8], BF16, tag="q8")
                    nc.vector.tensor_copy(q8[:], q8_ps[0:64, 0:8])
                    nc.sync.dma_start(
                        kv_ag_in[QOFF:HOFF].rearrange("(d m) -> d m", d=64),
                        q8[:])
                    hb = ap_.tile([P, 8], BF16, tag="hb")
                    nc.vector.tensor_copy(hb[:], h[:, :, 511])
                    nc.sync.dma_start(
                        kv_ag_in[HOFF:AGN].rearrange("(kt p) -> p kt", p=P),
                        hb[:])
                nc.gpsimd.collective_compute(
                    "AllGather", OP.bypass, replica_groups=PAIRS,
                    ins=[kv_ag_in[:]], outs=[kv_ag_out[:, :]])

                if not last:
                    qT = ap_.tile([P, 8, 512], BF16, tag="qT")
                    for ct in range(8):
                        wc = apw.tile([P, 8, P], BF16, name="wqkv_ct")
                        nc.sync.dma_start(wc[:], wqkv_d[ly, ct])
                        q_ps = ps_mm.tile([P, 512], F32, name="mm")
                        for kt in range(8):
                            nc.tensor.matmul(q_ps[:], wc[:, kt, :], xh[:, kt, :],
                                             start=(kt == 0), stop=(kt == 7))
                        nc.vector.tensor_copy(qT[:, ct, :], q_ps[:])

                # kT duplicated on both partition halves: [128, 4 h4, 1024 t]
                kTd = ap_.tile([P, 4, T], BF16, tag="kTd")
                for half in range(2):
                    src = kv_ag_out[half, KOFF:VOFF].rearrange(
                        "(h4 d t) -> d h4 t", h4=4, d=64)
                    nc.sync.dma_start(kTd[0:64, :, half * TL:(half + 1) * TL], src)
                    nc.sync.dma_start(kTd[64:128, :, half * TL:(half + 1) * TL], src)
                vst = ap_.tile([P, 8, 4, 65], BF16, tag="vst")
                nc.gpsimd.memset(vst[:, :, :, 64:65], 1.0)
                for hf in range(2):
                    for h4 in range(4):
                        nc.sync.dma_start(
                            vst[:, hf * 4:(hf + 1) * 4, h4, 0:64],
                            kv_ag_out[hf, VOFF:QOFF].rearrange(
                                "(tt p c) -> p tt c", p=P, tt=4)
                            [:, :, h4 * 64:(h4 + 1) * 64])

                if last:
                    _last_layer(nc, tc, pools, lctx, ap_, apw, ps_mm, ps_acc,
                                lnp, b3, h, kTd, vst, kv_ag_out,
                                wout3_d, wup3_d, wgate3_d, wdown3_d,
                                fin_ag_in, fin_ag_out, ly)
                    continue

                # ---- attention: 4 kv blocks x 2 units ----
                ost = [ap_.tile([P, 1024], BF16, tag=f"ost{r}", name=f"ost{r}")
                       for r in range(4)]
                for c in range(4):
                    o_ps = [[ps_acc.tile([P, 512], F32, name=f"acc{u * 2 + lh}")
                             for lh in range(2)] for u in range(2)]
                    for jt in range(8):
                        h4, tlt = jt // 2, jt % 2
                        tl0 = tlt * P
                        ta0 = 32 * tlt         # tri-skip: tau range [ta0, 64)
                        a_chunk = apw.tile([P, 2, 8, 2, 64], BF16, name="a_chunk")
                        for par in range(2):
                            b0 = par * 64
                            for hq in range(2):
                                s_ps = ps_mm.tile([P, 4, 2, 64], F32, name="mm")
                                # one matmul for all 4 heads of the hq group:
                                # same stationary k-tile, N=512/256
                                lhsT = kTd[b0:b0 + 64, h4,
                                           c * 256 + tl0: c * 256 + tl0 + P]
                                rhs = qT[b0:b0 + 64,
                                         hq * 4:(hq + 1) * 4, :].rearrange(
                                    "p h (blk tau) -> p h blk tau",
                                    tau=64)[:, :, c::4, ta0:64]
                                nc.tensor.matmul(s_ps[:, :, :, ta0:64],
                                                 lhsT, rhs,
                                                 start=True, stop=True)
                                nc.scalar.activation(
                                    a_chunk[:, par, hq * 4:(hq + 1) * 4, :,
                                            ta0:64],
                                    s_ps[:, :, :, ta0:64],
                                    AF.Exp, scale=0.125)
                                # mask right after each exp slice: keeps the
                                # o-matmuls from waiting on one big multiply
                                nc.vector.tensor_mul(
                                    a_chunk[:, par, hq * 4:(hq + 1) * 4, :,
                                            ta0:ta0 + 32],
                                    a_chunk[:, par, hq * 4:(hq + 1) * 4, :,
                                            ta0:ta0 + 32],
                                    masks[h4][:, par, hq * 4:(hq + 1) * 4, :, :])
                        tt8 = (c * 256 + tl0) // P
                        for u in range(2):
                            for lh in range(2):
                                if lh == 0 and tlt == 1:
                                    continue    # fully masked quarter
                                rhs = a_chunk[:, :, :, u, lh * 32:(lh + 1) * 32]
                                nc.tensor.matmul(
                                    o_ps[u][lh][0:65, :],
                                    vst[:, tt8, h4, :], rhs,
                                    start=(jt == 0),
                                    stop=(jt == 7 if lh else jt == 6))
                    # evacuate raw (frees PSUM fast); normalize afterwards
                    oraw = apw.tile([P, 4, 512], BF16, name="oraw")
                    for u in range(2):
                        for lh in range(2):
                            nc.vector.tensor_copy(oraw[0:65, u * 2 + lh, :],
                                                  o_ps[u][lh][0:65, :])
                    for u in range(2):
                        r = u * 2 + (c // 2)
                        for lh in range(2):
                            rcp = wk.tile([1, 512], F32, name="rcp")
                            nc.vector.reciprocal(rcp[:],
                                                 oraw[64:65, u * 2 + lh, :])
                            rcb = wk.tile([64, 512], F32, name="rcb")
                            nc.gpsimd.partition_broadcast(rcb[:], rcp[:])
                            nc.vector.tensor_mul(
                                ost[r][(c % 2) * 64:(c % 2) * 64 + 64,
                                       lh * 512:(lh + 1) * 512],
                                oraw[0:64, u * 2 + lh, :], rcb[:])

                # ---- out-projection + pair reduce-scatter ----
                for rout in range(8):
                    woc = apw.tile([P, 4, P], BF16, name="wocol")
                    nc.sync.dma_start(woc[:], wout_d[ly, rout])
                    for lh in range(2):
                        p_ps = ps_mm.tile([P, 512], F32, name="mm")
                        for kt in range(4):
                            rhs = ost[kt][:, lh * 512:(lh + 1) * 512].rearrange(
                                "p (par hidx tau) -> p tau hidx par",
                                par=2, hidx=8)
                            nc.tensor.matmul(p_ps[:], woc[:, kt, :], rhs,
                                             start=(kt == 0), stop=(kt == 3))
                        ap_sb = wk.tile([P, 512], BF16, name="ap_sb")
                        nc.vector.tensor_copy(ap_sb[:], p_ps[:])
                        nc.sync.dma_start(rs_in[lh, rout, :, :], ap_sb[:])
                nc.gpsimd.collective_compute(
                    "ReduceScatter", OP.add, replica_groups=PAIRS,
                    ins=[rs_in[:, :, :, :]], outs=[rs_out[:, :, :]])
                for r in range(8):
                    at = wk.tile([P, 512], BF16, name="at_sb")
                    nc.sync.dma_start(at[:], rs_out[r, :, :])
                    atf = wk.tile([P, 512], F32, name="atf_sb")
                    nc.vector.tensor_copy(atf[:], at[:])
                    nc.vector.tensor_add(h[:, r, :], h[:, r, :], atf[:])

            if last:
                continue
            # ---------------- FFN ----------------
            with ExitStack() as fctx:
                fp = fctx.enter_context(tc.tile_pool(name=f"ffn{ly}", bufs=1))
                fpw = fctx.enter_context(tc.tile_pool(name=f"ffnw{ly}", bufs=4))
                fps = fctx.enter_context(tc.tile_pool(name=f"ffns{ly}", bufs=2))
                x2 = fp.tile([P, 8, 512], BF16, tag="x2")
                _layer_norm(nc, pools,
                            [h[:, r, :] for r in range(8)],
                            [x2[:, r, :] for r in range(8)],
                            lambda r: lnp["ln2g"][:, ly, r:r + 1],
                            lambda r: lnp["ln2b"][:, ly, r:r + 1])
                hg = fp.tile([P, 32, 512], BF16, tag="hg")
                for ch in range(16):          # F chunks of 256
                    wu = fpw.tile([P, 8, 256], BF16, name="wup")
                    nc.sync.dma_start(wu[:], wup_d[ly, ch])
                    wg = fpw.tile([P, 8, 256], BF16, name="wgate")
                    nc.sync.dma_start(wg[:], wgate_d[ly, ch])
                    for fi in range(2):       # F-tiles of 128 in chunk
                        ft = ch * 2 + fi
                        u_ps = ps_mm.tile([P, 512], F32, name="mm")
                        for kt in range(8):
                            nc.tensor.matmul(u_ps[:], wu[:, kt, fi * P:(fi + 1) * P],
                                             x2[:, kt, :],
                                             start=(kt == 0), stop=(kt == 7))
                        g_ps = ps_mm.tile([P, 512], F32, name="mm")
                        for kt in range(8):
                            nc.tensor.matmul(g_ps[:], wg[:, kt, fi * P:(fi + 1) * P],
                                             x2[:, kt, :],
                                             start=(kt == 0), stop=(kt == 7))
                        u_sb = fps.tile([P, 512], BF16, name="u_sb")
                        nc.scalar.activation(u_sb[:], u_ps[:], AF.Identity,
                                             bias=lnp["bup"][:, ly, ft:ft + 1])
                        g_sb = fps.tile([P, 512], BF16, name="g_sb")
                        nc.scalar.activation(g_sb[:], g_ps[:], AF.Gelu_apprx_tanh,
                                             bias=lnp["bgate"][:, ly, ft:ft + 1])
                        nc.vector.tensor_mul(hg[:, ft, :], u_sb[:], g_sb[:])
                # down: 2 groups of 4 out-tiles, Wdown streamed per group
                for grp in range(2):
                    d_ps = [ps_acc.tile([P, 512], F32, name=f"acc{i}")
                            for i in range(4)]
                    for kt in range(32):
                        wd = fpw.tile([P, 1024], BF16, name="wdown")
                        nc.sync.dma_start(wd[:],
                                          wdown_d[ly, kt * P:(kt + 1) * P, :])
                        for i in range(4):
                            rout = grp * 4 + i
                            nc.tensor.matmul(d_ps[i][:],
                                             wd[:, rout * P:(rout + 1) * P],
                                             hg[:, kt, :],
                                             start=(kt == 0), stop=(kt == 31))
                    for i in range(4):
                        rout = grp * 4 + i
                        dn = fps.tile([P, 512], F32, name="dn_sb")
                        nc.scalar.activation(dn[:], d_ps[i][:], AF.Identity,
                                             bias=lnp["bdown"][:, ly, rout:rout + 1])
                        nc.vector.tensor_add(h[:, rout, :], h[:, rout, :], dn[:])

        # -------- tail: assemble h_fin for 4 batches, final LN, LM head ------
        with ExitStack() as tctx:
            tp = tctx.enter_context(tc.tile_pool(name="tail", bufs=6))
            he = tp.tile([P, 8, 4, 2], F32, name="he")
            for bb in range(4):
                for s in range(2):
                    nc.gpsimd.dma_start(
                        he[:, :, bb, s],
                        fin_ag_out[2 * bb + s].rearrange("(r p) -> p r", p=P))
            hfin4 = tp.tile([P, 8, 4], F32R, name="hfin4")
            nc.vector.tensor_add(hfin4[:], he[:, :, :, 0], he[:, :, :, 1])
            hall = tp.tile([P, 8, 4], BF16, name="hall")
            _vec_ln(nc, pools, tp, hfin4,
                    lambda r: hall[:, r, :],
                    lambda r: fln[:, 0, r:r + 1],
                    lambda r: fln[:, 1, r:r + 1], 4)
            for nt in range(8):
                n0, n1 = nt * 500, (nt + 1) * 500
                l_ps = ps_mm.tile([P, 512], F32, name="mm")
                for kt in range(8):
                    if n1 <= 2000:
                        wl_ap = wlmA[:, kt, n0:n1]
                    else:
                        wl = tp.tile([P, 500], BF16, name="wlm")
                        nc.sync.dma_start(wl[:], wlm_d[:, kt, n0:n1])
                        wl_ap = wl[:]
                    nc.tensor.matmul(l_ps[0:4, 0:500], hall[:, kt, :], wl_ap,
                                     start=(kt == 0), stop=(kt == 7))
                bl = tp.tile([1, 500], F32, name="blm")
                nc.sync.dma_start(bl[:], blm_d[n0:n1])
                blb = tp.tile([4, 500], F32, name="blb")
                nc.gpsimd.partition_broadcast(blb[:], bl[:])
                lo = tp.tile([4, 512], F32, name="lo")
                nc.vector.tensor_add(lo[:, 0:500], l_ps[0:4, 0:500], blb[:])
                nc.sync.dma_start(logits_d[:, n0:n1], lo[:, 0:500])

    nc.compile()
    return nc


def _last_layer(nc, tc, pools, lctx, ap_, apw, ps_mm, ps_acc, lnp, b3, h,
                kTd, vst, kv_ag_out, wout3_d, wup3_d, wgate3_d, wdown3_d,
                fin_ag_in, fin_ag_out, ly):
    """Layer 3: only token 1023 survives -> 1-token attention + out-proj
    (duplicated on both pair cores) and a pair-F-sharded 1-token FFN."""
    wk = pools["wk"]
    # q rows for all 16 units: [64 d, 16 m]
    qa = ap_.tile([64, 16], BF16, tag="qa")
    for s in range(2):
        nc.sync.dma_start(
            qa[0:64, s * 8:(s + 1) * 8],
            kv_ag_out[s, QOFF:HOFF].rearrange("(d m) -> d m", d=64))
    h1023b = ap_.tile([P, 8], BF16, tag="h1023b")
    nc.sync.dma_start(h1023b[:],
                      kv_ag_out[1, HOFF:AGN].rearrange("(kt p) -> p kt", p=P))

    # 1-token attention, no mask (l=1023 attends everything)
    of = ap_.tile([64, 4, 4], BF16, tag="of")      # [d, c, mi]
    for c in range(4):
        s_ps = ps_mm.tile([P, 512], F32, name="mm")
        for th in range(2):
            for h4q in range(4):
                col = (th * 4 + h4q) * 4
                lhsT = kTd[0:64, h4q,
                           c * 256 + th * P: c * 256 + (th + 1) * P]
                nc.tensor.matmul(s_ps[:, col:col + 4], lhsT, qa[0:64, c::4],
                                 start=True, stop=True)
        a1 = apw.tile([P, 32], BF16, name="a1")
        nc.scalar.activation(a1[:], s_ps[:, 0:32], AF.Exp, scale=0.125)
        o1_ps = ps_acc.tile([P, 512], F32, name="acc0")
        for th in range(2):
            for h4q in range(4):
                col = (th * 4 + h4q) * 4
                nc.tensor.matmul(o1_ps[0:65, 0:4],
                                 vst[:, c * 2 + th, h4q, :],
                                 a1[:, col:col + 4],
                                 start=(th == 0 and h4q == 0),
                                 stop=(th == 1 and h4q == 3))
        rcp1 = wk.tile([1, 512], F32, name="rcp")
        nc.vector.reciprocal(rcp1[0:1, 0:4], o1_ps[64:65, 0:4])
        rcb1 = wk.tile([64, 512], F32, name="rcb")
        nc.gpsimd.partition_broadcast(rcb1[0:64, 0:4], rcp1[0:1, 0:4])
        nc.vector.tensor_mul(of[:, c, :], o1_ps[0:64, 0:4], rcb1[0:64, 0:4])

    # assemble o_final^T: chan = m*64 + d -> ofT[(m%2)*64 + d, m//2]
    ofT = ap_.tile([P, 8], BF16, tag="ofT")
    for c in range(4):
        for mi in range(4):
            m = mi * 4 + c
            p0 = (m % 2) * 64
            nc.vector.tensor_copy(ofT[p0:p0 + 64, m // 2:m // 2 + 1],
                                  of[:, c, mi:mi + 1])

    # 1-token out-projection (full contraction, duplicated on both cores)
    ya = ap_.tile([P, 8], F32, tag="ya")
    for r_out in range(8):
        woc3 = apw.tile([P, 8, P], BF16, name="woc3")
        nc.sync.dma_start(woc3[:], wout3_d[r_out])
        y_ps = ps_mm.tile([P, 512], F32, name="mm")
        for kt in range(8):
            nc.tensor.matmul(y_ps[:, 0:1], woc3[:, kt, :], ofT[:, kt:kt + 1],
                             start=(kt == 0), stop=(kt == 7))
        nc.vector.tensor_copy(ya[:, r_out:r_out + 1], y_ps[:, 0:1])

    h23 = ap_.tile([P, 8], F32, tag="h23")
    h1023f = ap_.tile([P, 8], F32, tag="h1023f")
    nc.vector.tensor_copy(h1023f[:], h1023b[:])
    nc.vector.tensor_add(h23[:], h1023f[:], ya[:])

    # LN2 on the single token (duplicate the column for even-N f32r matmuls)
    ht2 = ap_.tile([P, 8, 2], F32R, tag="ht2")
    nc.vector.tensor_copy(ht2[:, :, 0], h23[:])
    nc.vector.tensor_copy(ht2[:, :, 1], h23[:])
    x2t = ap_.tile([P, 8, 2], BF16, tag="x2t")
    _vec_ln(nc, pools, apw, ht2,
            lambda r: x2t[:, r, :],
            lambda r: lnp["ln2g"][:, ly, r:r + 1],
            lambda r: lnp["ln2b"][:, ly, r:r + 1], 2)

    # FFN on my F-half (2048 features), feature-major
    hg3 = ap_.tile([P, 16], BF16, tag="hg3")
    for fc in range(16):
        wu3 = apw.tile([P, 8, P], BF16, name="wu3")
        nc.sync.dma_start(wu3[:], wup3_d[fc])
        wg3 = apw.tile([P, 8, P], BF16, name="wg3")
        nc.sync.dma_start(wg3[:], wgate3_d[fc])
        u_ps = ps_mm.tile([P, 512], F32, name="mm")
        for kt in range(8):
            nc.tensor.matmul(u_ps[:, 0:1], wu3[:, kt, :], x2t[:, kt, 0:1],
                             start=(kt == 0), stop=(kt == 7))
        g_ps = ps_mm.tile([P, 512], F32, name="mm")
        for kt in range(8):
            nc.tensor.matmul(g_ps[:, 0:1], wg3[:, kt, :], x2t[:, kt, 0:1],
                             start=(kt == 0), stop=(kt == 7))
        u_sb = wk.tile([P, 1], BF16, name="u3_sb")
        nc.scalar.activation(u_sb[:], u_ps[:, 0:1], AF.Identity,
                             bias=b3[:, 0, fc:fc + 1])
        g_sb = wk.tile([P, 1], BF16, name="g3_sb")
        nc.scalar.activation(g_sb[:], g_ps[:, 0:1], AF.Gelu_apprx_tanh,
                             bias=b3[:, 1, fc:fc + 1])
        nc.vector.tensor_mul(hg3[:, fc:fc + 1], u_sb[:], g_sb[:])
    y3_ps = ps_acc.tile([P, 512], F32, name="acc1")
    for dc in range(8):
        wd3 = apw.tile([P, 16, P], BF16, name="wd3")
        nc.sync.dma_start(wd3[:], wdown3_d[dc])
        for fk in range(16):
            nc.tensor.matmul(y3_ps[:, dc:dc + 1], wd3[:, fk, :],
                             hg3[:, fk:fk + 1],
                             start=(fk == 0), stop=(fk == 15))
    # s_fin = 0.5*(h23 + bdown) + y3_partial ; pair sums in the final AG
    tmp = ap_.tile([P, 8], F32, tag="sfin_tmp")
    nc.vector.tensor_add(tmp[:], h23[:], lnp["bdown"][:, ly, :])
    y3_sb = ap_.tile([P, 8], F32, tag="y3_sb")
    nc.vector.tensor_copy(y3_sb[:], y3_ps[:, 0:8])
    s_fin = ap_.tile([P, 8], F32, tag="s_fin")
    nc.vector.scalar_tensor_tensor(s_fin[:], tmp[:], 0.5, y3_sb[:],
                                   op0=OP.mult, op1=OP.add)
    nc.sync.dma_start(fin_ag_in.ap().rearrange("(r p) -> p r", p=P), s_fin[:])
    nc.gpsimd.collective_compute(
        "AllGather", OP.bypass, replica_groups=ALL8,
        ins=[fin_ag_in[:]], outs=[fin_ag_out[:, :]])


def _pe_table(t, d):
    pos = np.arange(t, dtype=np.float32)[:, None]
    freq = np.exp(-(np.arange(0, d, 2, dtype=np.float32) / d) * np.log(10000.0))
    ang = pos * freq[None, :]
    pe = np.zeros((t, d), dtype=np.float32)
    pe[:, 0::2] = np.sin(ang)
    pe[:, 1::2] = np.cos(ang)
    return pe


def _prepack(Wqkv, Wup, Wgate, Wdown, Wlm, nl):
    """Host-side bf16 prepack into per-tile DMA layouts."""
    bf = lambda x: np.ascontiguousarray(x).astype(NPBF16)
    wq = Wqkv[:, :, :1280].reshape(nl, 8, P, 10, P)      # [l, kt, p, ct, c]
    wqkvP = bf(wq.transpose(0, 3, 2, 1, 4))
    wvP = bf(Wqkv[:, :, 1280:1536].reshape(nl, 8, P, 256).transpose(0, 2, 1, 3))
    wupP = bf(Wup.reshape(nl, 8, P, 16, 256).transpose(0, 3, 2, 1, 4))
    wgateP = bf(Wgate.reshape(nl, 8, P, 16, 256).transpose(0, 3, 2, 1, 4))
    wdownP = bf(Wdown)
    wlmP = bf(Wlm.reshape(8, P, V).transpose(1, 0, 2))   # [p, kt, V]
    return wqkvP, wvP, wupP, wgateP, wdownP, wlmP


def kernel(idx, emb, Wqkv, Wout, ln1_g, ln1_b, ln2_g, ln2_b, Wup, bup,
           Wgate, bgate, Wdown, bdown, fln_g, fln_b, Wlm, blm, _trace=False):
    f32 = lambda x: np.ascontiguousarray(np.asarray(x, dtype=np.float32))
    bf = lambda x: np.ascontiguousarray(np.asarray(x)).astype(NPBF16)
    idx = np.asarray(idx)
    emb = f32(emb)
    Wqkv, Wout, Wup, Wgate, Wdown, Wlm = map(
        lambda x: np.asarray(x, dtype=np.float32),
        (Wqkv, Wout, Wup, Wgate, Wdown, Wlm))
    blm_f = f32(blm)
    bup_f, bgate_f = f32(bup), f32(bgate)

    nl = int(Wqkv.shape[0])
    if ("nc", nl) not in _CACHE:
        _CACHE[("nc", nl)] = build_kernel(nl)
    nc = _CACHE[("nc", nl)]

    wqkvP, wvP, wupP, wgateP, wdownP, wlmP = _prepack(
        Wqkv, Wup, Wgate, Wdown, Wlm, nl)
    wout3P = bf(Wout[nl - 1].reshape(8, P, 8, P).transpose(2, 1, 0, 3))

    pe = _pe_table(T, D)
    h0 = emb[np.asarray(idx)] * np.float32(np.sqrt(D)) + pe[None]  # [B, T, D]

    in_maps = []
    for core in range(8):
        b, half = core // 2, core % 2
        t0 = half * TL
        wo = Wout[:, t0:t0 + TL, :].reshape(nl, 4, P, 8, P)
        woutP = bf(wo.transpose(0, 3, 2, 1, 4))
        fsl = slice(half * 16, (half + 1) * 16)
        wup3P = bf(Wup[nl - 1].reshape(8, P, 32, P)[:, :, fsl, :]
                   .transpose(2, 1, 0, 3))
        wgate3P = bf(Wgate[nl - 1].reshape(8, P, 32, P)[:, :, fsl, :]
                     .transpose(2, 1, 0, 3))
        wdown3P = bf(Wdown[nl - 1].reshape(32, P, 8, P)[fsl, :, :, :]
                     .transpose(2, 1, 0, 3))
        bup3P = np.ascontiguousarray(
            bup_f[nl - 1].reshape(32, P)[fsl].T)
        bgate3P = np.ascontiguousarray(
            bgate_f[nl - 1].reshape(32, P)[fsl].T)
        in_maps.append({
            "h0t": np.ascontiguousarray(h0[b, t0:t0 + TL].T),
            "wqkv": wqkvP, "wv": wvP,
            "wout": woutP, "wout3": wout3P,
            "wup": wupP, "wgate": wgateP, "wdown": wdownP,
            "wup3": wup3P, "wgate3": wgate3P, "wdown3": wdown3P,
            "bup3": bup3P, "bgate3": bgate3P,
            "ln1g": f32(ln1_g), "ln1b": f32(ln1_b),
            "ln2g": f32(ln2_g), "ln2b": f32(ln2_b),
            "bup": bup_f, "bgate": bgate_f, "bdown": f32(bdown),
            "flng": f32(fln_g), "flnb": f32(fln_b),
            "wlm": np.ascontiguousarray(wlmP[:, :, core * VC:(core + 1) * VC]),
            "blm": np.ascontiguousarray(blm_f[core * VC:(core + 1) * VC]),
        })
    res = run_bass_kernel_spmd(nc, in_maps, core_ids=list(range(8)),
                               trace=_trace)
    logits = np.zeros((B, 1, V), dtype=np.float32)
    for core in range(8):
        logits[:, 0, core * VC:(core + 1) * VC] = res.results[core]["logits"]
    if _trace:
        return logits, res
    return logits
